# revision 46
# baseline (speedup 1.0000x reference)
"""MoE block (N=8192, D=1024, H=4096, E=8, top_k=2) on 8 Trainium2 NeuronCores.

Strategy
--------
Expert-parallel FFN + data-parallel combine, two SPMD launches.

Launch 1 (FFN): the 16384 (token, expert) pairs are packed into a uniform
per-core grid of 6 single-expert chunks (5x384 + 1x256 slots = 2176/core).
Chunk capacities are identical on every core (SPMD), and each chunk's
expert weights are shipped per-core in chunk-slot order, so no core ever
re-streams a weight matrix and every matmul2 token tile is full-width.
Both matmuls run in fp8-e4m3 with the TensorEngine's DoubleRow perf mode;
all quantization scales are powers of two folded host-side into b1, the
combine weights, and one constant. Outputs: w_k-scaled y rows in slot
order (bf16).

Host between launches: pure permutation of y rows into token order.

Launch 2 (combine): data-parallel over tokens; per 128-token tile, load
the K=2 weighted expert rows + residual (with b2 folded in host-side),
two adds on Pool/Vector, then LayerNorm.

Measured end-to-end relative error ~8e-3 (gate: 2e-2).
"""

import os
import sys

import numpy as np

for _p in ("/opt/trn_rl_repo", "/root/.axon_site/_ro/trn_rl_repo"):
    if os.path.isdir(_p) and _p not in sys.path:
        sys.path.append(_p)

import ml_dtypes

import concourse.bass as bass
import concourse.mybir as mybir
import concourse.tile as tile
from concourse import bacc
from concourse.bass_utils import run_bass_kernel_spmd

FP8 = mybir.dt.float8e4
BF16 = mybir.dt.bfloat16
F32 = mybir.dt.float32
NP_FP8 = ml_dtypes.float8_e4m3
NP_BF16 = ml_dtypes.bfloat16
DOUBLE_ROW = mybir.MatmulPerfMode.DoubleRow

P = 128          # SBUF partitions
MMAX = 512       # max moving free dim / fp32 PSUM bank
BIG, SMALL = 384, 256   # uniform chunk grid: 5 BIG + 1 SMALL per core
LN_EPS = 1e-5
N_CORES = 8

# fp8 quantization scales (powers of two; e4m3 max finite = 240)
SX = 16.0   # x ~ N(0,1), max |x| ~ 4.5 -> 72
SW1 = 64.0  # w1 ~ U(-1/32, 1/32) -> 2
SH = 16.0   # h = relu(...) max ~ 4 -> 64
SW2 = 64.0  # w2 ~ U(-1/64, 1/64) -> 1
MM1_EVAC_SCALE = SH / (SX * SW1)


# ---------------------------------------------------------------- host routing

def _softmax(z, axis=-1):
    z = z - z.max(axis=axis, keepdims=True)
    ez = np.exp(z)
    return ez / ez.sum(axis=axis, keepdims=True)


def _route(x, gate_w, gate_b, top_k):
    """fp64 gating. Returns topk idx [N,K] and renormalized weights [N,K] f32."""
    logits = x.astype(np.float64) @ gate_w.astype(np.float64).T + gate_b.astype(
        np.float64
    )
    p = _softmax(logits)
    # stable argsort of -p == jax.lax.top_k tie-breaking (lower index first)
    topk = np.argsort(-p, axis=-1, kind="stable")[:, :top_k]
    ps = np.take_along_axis(p, topk, axis=1)
    w = _softmax(ps).astype(np.float32)
    return topk, w


def _pack_experts(loads, n_big, n_small):
    """Per-expert (bigs, smalls) chunk counts covering each load, fitting the
    global supply. Brute force over a small option window per expert."""
    import itertools

    opts = []
    for ld in loads:
        fl = int(ld) // BIG
        o = []
        for a in range(max(0, fl - 2), fl + 3):
            need = int(ld) - a * BIG
            b = 0 if need <= 0 else -(-need // SMALL)
            if b <= n_small:
                o.append((a, b))
        opts.append(o)
    best = None
    for combo in itertools.product(*opts):
        sa = sum(a for a, _ in combo)
        sb = sum(b for _, b in combo)
        if sa <= n_big and sb <= n_small:
            key = (sa + sb, sb)
            if best is None or key < best[0]:
                best = (key, combo)
    assert best is not None, "chunk packing infeasible"
    return best[1]


# ------------------------------------------------------------- device programs

def _build_ffn(D, H, NCH, slot_sizes, offs, CT):
    """Launch 1: uniform expert-chunk FFN. NCH chunks per core."""
    nc = bacc.Bacc()

    nDP = D // (2 * P)    # DoubleRow contraction pairs for matmul1 (4)
    nH = H // P           # h blocks (32)
    nHP = nH // 2         # h pairs for matmul2 contraction (16)
    nDC = (D + MMAX - 1) // MMAX

    xg_d = nc.dram_tensor("xg", [nDP, P, 2, CT], FP8, kind="ExternalInput")
    w1c_d = nc.dram_tensor("w1c", [NCH, nDP, P, 2, H], FP8, kind="ExternalInput")
    w2c_d = nc.dram_tensor("w2c", [NCH, nHP, P, 2, D], FP8, kind="ExternalInput")
    # pre-transposed on host: b1c[p, ci*nH+hb] = b1[expert(ci), hb*P+p]
    b1c_d = nc.dram_tensor("b1c", [P, NCH * nH], F32, kind="ExternalInput")
    wexp_d = nc.dram_tensor("wexp", [CT + P], F32, kind="ExternalInput")
    yb_d = nc.dram_tensor("yb", [CT, D], BF16, kind="ExternalOutput")

    with tile.TileContext(nc) as tc:
        with (
            tc.tile_pool(name="consts", bufs=1) as consts,
            tc.tile_pool(name="w1p", bufs=8) as w1p,
            tc.tile_pool(name="w2p", bufs=28) as w2p,
            tc.tile_pool(name="xgp", bufs=2 * nDP) as xgp,
            tc.tile_pool(name="htp", bufs=6) as htp,
            tc.tile_pool(name="yp", bufs=4) as yp,
            tc.tile_pool(name="sp", bufs=4) as sp,
            tc.tile_pool(name="php", bufs=2, space="PSUM") as php,
            tc.tile_pool(name="pyp", bufs=6, space="PSUM") as pyp,
        ):
            # b1 for all chunks, h-on-partition layout (host pre-transposed:
            # a plain 2D load, not a 26k-descriptor element-wise transpose)
            b1a_t = consts.tile([P, NCH * nH], F32)
            nc.sync.dma_start(out=b1a_t, in_=b1c_d[:, :])

            def load_xg_wx(ci, engs=None):
                off, C = offs[ci], slot_sizes[ci]
                ntt = (C + P - 1) // P
                xg_t = []
                for dp in range(nDP):
                    t = xgp.tile([P, 2, C], FP8, tag="xg", name=f"xg_{ci}_{dp}")
                    # steady state issues from GpSimd: the Sync sequencer is
                    # otherwise the DMA-issue bottleneck (~98% busy)
                    eng = engs[dp % len(engs)] if engs else nc.gpsimd
                    eng.dma_start(out=t, in_=xg_d[dp, :, :, off : off + C])
                    xg_t.append(t)
                wx_t = sp.tile([P, ntt], F32, tag="wx")
                _l = wexp_d[off : off + C]
                nc.gpsimd.dma_start(
                    out=wx_t[:, :],
                    in_=bass.AP(
                        tensor=_l.tensor, offset=_l.offset, ap=[[1, P], [P, ntt]]
                    ),
                )
                return xg_t, wx_t

            def alloc_w1(ci):
                return [
                    w1p.tile([P, 2, H], FP8, tag="w1", name=f"w1_{ci}_{dp}")
                    for dp in range(nDP)
                ]

            def load_w1_part(ci, w1_t, part, eng=None):
                # q-major: all four dp slices of quarter 0 land first, so the
                # first h-blocks' matmuls can start after ~1MB, not ~3.4MB
                q, dp = divmod(part, nDP)
                hq = H // nDP
                eng = eng or (nc.sync if part % 2 else nc.gpsimd)
                eng.dma_start(
                    out=w1_t[dp][:, :, q * hq : (q + 1) * hq],
                    in_=w1c_d[ci, dp, :, :, q * hq : (q + 1) * hq],
                )

            # chunk-0 startup: spread DMA issue over three engine sequencers
            # so descriptor generation isn't serialized on Sync
            startup_engs = [nc.sync, nc.scalar, nc.gpsimd]
            pend_xg = load_xg_wx(0, engs=startup_engs)
            pend_w1 = alloc_w1(0)
            for part in range(nHP):
                load_w1_part(0, pend_w1, part, eng=startup_engs[part % 3])

            for ci in range(NCH):
                off, C = offs[ci], slot_sizes[ci]
                ntt = (C + P - 1) // P
                xg_t, wx_t = pend_xg
                w1_t = pend_w1
                nxt = ci + 1 if ci + 1 < NCH else None
                if nxt is not None:
                    pend_w1 = alloc_w1(nxt)

                w2_t = []
                for hp in range(nHP):
                    t = w2p.tile([P, 2, D], FP8, tag="w2")
                    nc.gpsimd.dma_start(out=t, in_=w2c_d[ci, hp])
                    w2_t.append(t)
                    if nxt is not None:
                        if hp == 0:
                            pend_xg = load_xg_wx(nxt)
                        load_w1_part(nxt, pend_w1, hp)

                py = {}
                for tt in range(ntt):
                    for ch in range(nDC):
                        pt = pyp.tile([P, MMAX], F32, tag="py")
                        py[(tt, ch)] = pt

                ht_t = {}

                def mm2_step(hp, py=py, ht_t=ht_t, w2_t=w2_t, C=C, ntt=ntt):
                    for tt in range(ntt):
                        s = tt * P
                        mw = min(P, C - s)
                        for ch in range(nDC):
                            d0 = ch * MMAX
                            dw = min(MMAX, D - d0)
                            nc.tensor.matmul(
                                py[(tt, ch)][:mw, :dw],
                                ht_t[hp][:, :, s : s + mw],
                                w2_t[hp][:, :, d0 : d0 + dw],
                                start=(hp == 0),
                                stop=(hp == nHP - 1),
                                perf_mode=DOUBLE_ROW,
                            )

                for hb in range(nH):
                    hp, j = hb // 2, hb % 2
                    if j == 0:
                        ht = htp.tile([P, 2, C], FP8, tag="ht")
                        ht_t[hp] = ht
                    ph = php.tile([P, MMAX], F32, tag="ph")
                    for dp in range(nDP):
                        nc.tensor.matmul(
                            ph[:, :C],
                            w1_t[dp][:, :, hb * P : (hb + 1) * P],
                            xg_t[dp][:, :, :],
                            start=(dp == 0),
                            stop=(dp == nDP - 1),
                            perf_mode=DOUBLE_ROW,
                        )
                    nc.scalar.activation(
                        out=ht_t[hp][:, j, :],
                        in_=ph[:, :C],
                        func=mybir.ActivationFunctionType.Relu,
                        bias=b1a_t[:, ci * nH + hb : ci * nH + hb + 1],
                        scale=MM1_EVAC_SCALE,
                    )
                    if j == 1 and hp >= 3:
                        mm2_step(hp - 3)
                mm2_step(nHP - 3)
                mm2_step(nHP - 2)
                mm2_step(nHP - 1)

                # evacuate y on the Vector engine, scaled by the combine
                # weight, straight to slot order (host permutes afterwards)
                for tt in range(ntt):
                    s = tt * P
                    mw = min(P, C - s)
                    yt = yp.tile([P, D], BF16, tag="y")
                    for ch in range(nDC):
                        d0 = ch * MMAX
                        dw = min(MMAX, D - d0)
                        nc.vector.tensor_scalar_mul(
                            yt[:mw, d0 : d0 + dw],
                            py[(tt, ch)][:mw, :dw],
                            wx_t[:mw, tt : tt + 1],
                        )
                    nc.sync.dma_start(
                        out=yb_d[off + s : off + s + mw, :], in_=yt[:mw]
                    )

    return nc


def _build_combine(D, NT, K, ln_identity):
    """Launch 2: residual + top-k combine + LayerNorm, data-parallel."""
    nc = bacc.Bacc()

    xr_d = nc.dram_tensor("xr", [NT, D], BF16, kind="ExternalInput")
    yg_d = nc.dram_tensor("yg", [NT, K, D], BF16, kind="ExternalInput")
    lnw_d = nc.dram_tensor("lnw", [D], F32, kind="ExternalInput")
    lnb_d = nc.dram_tensor("lnb", [D], F32, kind="ExternalInput")
    out_d = nc.dram_tensor("out", [NT, D], F32, kind="ExternalOutput")

    with tile.TileContext(nc) as tc:
        with (
            tc.tile_pool(name="consts", bufs=1) as consts,
            tc.tile_pool(name="cp", bufs=8) as cp,
            tc.tile_pool(name="ygp", bufs=8) as ygp,
            tc.tile_pool(name="sp", bufs=8) as sp,
        ):
            eps_t = consts.tile([P, 1], F32)
            nc.vector.memset(eps_t, LN_EPS)
            if not ln_identity:
                lnw_t = consts.tile([P, D], F32)
                _l = lnw_d[:]
                nc.sync.dma_start(
                    out=lnw_t,
                    in_=bass.AP(
                        tensor=_l.tensor, offset=_l.offset, ap=[[0, P], [1, D]]
                    ),
                )
                lnb_t = consts.tile([P, D], F32)
                _l = lnb_d[:]
                nc.sync.dma_start(
                    out=lnb_t,
                    in_=bass.AP(
                        tensor=_l.tensor, offset=_l.offset, ap=[[0, P], [1, D]]
                    ),
                )

            for t in range(NT // P):
                r0 = t * P
                xr_t = cp.tile([P, D], BF16, tag="xr")
                nc.scalar.dma_start(out=xr_t, in_=xr_d[r0 : r0 + P, :])
                yg = ygp.tile([P, K, D], BF16, tag="yg")
                nc.gpsimd.dma_start(out=yg, in_=yg_d[r0 : r0 + P])
                # adds split by half-columns and pipelined across Pool/DVE,
                # with each half's LN stats emitted as soon as it's ready
                ys = cp.tile([P, D], F32, tag="ys")
                acc = cp.tile([P, D], F32, tag="acc")
                hD = D // 2
                nsub = (D + 511) // 512
                assert nsub == 2 and hD == 512
                st = sp.tile([P, nsub, 6], F32, tag="st")
                for s, h0 in enumerate((0, hD)):
                    nc.gpsimd.tensor_add(
                        ys[:, h0 : h0 + hD],
                        yg[:, 0, h0 : h0 + hD],
                        yg[:, 1, h0 : h0 + hD],
                    )
                    nc.vector.tensor_add(
                        acc[:, h0 : h0 + hD],
                        xr_t[:, h0 : h0 + hD],
                        ys[:, h0 : h0 + hD],
                    )
                    nc.vector.bn_stats(
                        out=st[:, s, :], in_=acc[:, h0 : h0 + hD]
                    )
                mv = sp.tile([P, 2], F32, tag="mv")
                nc.vector.bn_aggr(out=mv, in_=st)
                nc.scalar.activation(
                    out=mv[:, 1:2],
                    in_=mv[:, 1:2],
                    func=mybir.ActivationFunctionType.Sqrt,
                    bias=eps_t[:, 0:1],
                )
                nc.vector.reciprocal(out=mv[:, 1:2], in_=mv[:, 1:2])
                nb = sp.tile([P, 1], F32, tag="nb")
                nc.vector.tensor_scalar(
                    out=nb,
                    in0=mv[:, 0:1],
                    scalar1=mv[:, 1:2],
                    scalar2=-1.0,
                    op0=mybir.AluOpType.mult,
                    op1=mybir.AluOpType.mult,
                )
                nc.scalar.activation(
                    out=acc,
                    in_=acc,
                    func=mybir.ActivationFunctionType.Identity,
                    scale=mv[:, 1:2],
                    bias=nb[:, 0:1],
                )
                if not ln_identity:
                    nc.vector.tensor_mul(acc, acc, lnw_t)
                    nc.vector.tensor_add(acc, acc, lnb_t)
                nc.sync.dma_start(out=out_d[r0 : r0 + P, :], in_=acc)

    return nc


# ----------------------------------------------------------------- entrypoint

def kernel(x, gate_w, gate_b, w1, b1, w2, b2, ln_w, ln_b, top_k):
    x = np.asarray(x, np.float32)
    w1 = np.asarray(w1, np.float32)
    b1 = np.asarray(b1, np.float32)
    w2 = np.asarray(w2, np.float32)
    b2 = np.asarray(b2, np.float32)
    ln_w = np.asarray(ln_w, np.float32)
    ln_b = np.asarray(ln_b, np.float32)
    K = int(top_k)

    N, D = x.shape
    E, H, _ = w1.shape
    NT = N // N_CORES
    assert N % (N_CORES * P) == 0 and D % (2 * P) == 0 and H % (2 * P) == 0

    topk, wts = _route(x, np.asarray(gate_w, np.float32), np.asarray(gate_b, np.float32), K)

    # ---- pack (token, expert) pairs into the uniform chunk grid
    loads = np.bincount(topk.ravel(), minlength=E)
    n_big, n_small = 5 * N_CORES, N_CORES
    ab = _pack_experts(loads, n_big, n_small)

    slot_sizes = [BIG] * 5 + [SMALL]
    offs = np.concatenate([[0], np.cumsum(slot_sizes)]).astype(np.int64)
    CT = int(offs[-1])
    NCH = len(slot_sizes)

    # global chunk list: (expert, pair_lo, pair_hi) in expert-major order;
    # -1 expert = dead filler chunk
    big_list, small_list = [], []
    pair_tok = [np.where((topk == e).any(axis=1))[0] for e in range(E)]
    pair_kk = [np.argmax(topk[pt] == e, axis=1) for e, pt in enumerate(pair_tok)]
    for e in range(E):
        a_e, b_e = ab[e]
        pos = 0
        for _ in range(a_e):
            big_list.append((e, pos, min(pos + BIG, int(loads[e]))))
            pos = min(pos + BIG, int(loads[e]))
        for _ in range(b_e):
            small_list.append((e, pos, min(pos + SMALL, int(loads[e]))))
            pos = min(pos + SMALL, int(loads[e]))
        assert pos >= loads[e]
    big_list += [(-1, 0, 0)] * (n_big - len(big_list))
    small_list += [(-1, 0, 0)] * (n_small - len(small_list))

    nDP, nHP = D // (2 * P), H // (2 * P)
    w1t = np.ascontiguousarray(
        (w1.transpose(0, 2, 1) * SW1)
        .astype(NP_FP8)
        .reshape(E, nDP, 2, P, H)
        .transpose(0, 1, 3, 2, 4)
    )
    w2t = np.ascontiguousarray(
        (w2.transpose(0, 2, 1) * SW2)
        .astype(NP_FP8)
        .reshape(E, nHP, 2, P, D)
        .transpose(0, 1, 3, 2, 4)
    )
    b1s = (b1 * SH).astype(np.float32)
    xq_all = (x * SX).astype(NP_FP8)  # [N, D]

    in_maps1 = []
    # permutation: for each core/slot -> destination row (token*K + k)
    dest_rows = []
    for c in range(N_CORES):
        core_chunks = big_list[5 * c : 5 * c + 5] + [small_list[c]]
        xgbuf = np.zeros((nDP, P, 2, CT), NP_FP8)
        w1c = np.zeros((NCH, nDP, P, 2, H), NP_FP8)
        w2c = np.zeros((NCH, nHP, P, 2, D), NP_FP8)
        b1c = np.zeros((NCH, H), np.float32)  # transposed to [P, NCH*nH] below
        wexp = np.zeros(CT + P, np.float32)
        drow = np.full(CT, -1, np.int64)
        for ci, (e, lo, hi) in enumerate(core_chunks):
            if e < 0 or hi <= lo:
                continue
            toks = pair_tok[e][lo:hi]
            kks = pair_kk[e][lo:hi]
            off = int(offs[ci])
            n_p = hi - lo
            xgbuf[:, :, :, off : off + n_p] = (
                xq_all[toks].T.reshape(nDP, 2, P, n_p).transpose(0, 2, 1, 3)
            )
            w1c[ci] = w1t[e]
            w2c[ci] = w2t[e]
            b1c[ci] = b1s[e]
            wexp[off : off + n_p] = wts[toks, kks] / (SH * SW2)
            drow[off : off + n_p] = toks * K + kks
        dest_rows.append(drow)
        nH = H // P
        b1ct = np.ascontiguousarray(
            b1c.reshape(NCH, nH, P).transpose(2, 0, 1).reshape(P, NCH * nH)
        )
        in_maps1.append(
            {"xg": xgbuf, "w1c": w1c, "w2c": w2c, "b1c": b1ct, "wexp": wexp}
        )

    nc1 = _build_ffn(D, H, NCH, slot_sizes, offs, CT)
    nc1.finalize()

    trace = os.environ.get("MOE_KERNEL_TRACE", "0") == "1"
    res1 = run_bass_kernel_spmd(nc1, in_maps1, list(range(N_CORES)), trace=trace)

    # ---- host: permute y rows into token order (pure data movement)
    yrows = np.zeros((N * K, D), NP_BF16)
    for c in range(N_CORES):
        dr = dest_rows[c]
        real = dr >= 0
        yrows[dr[real]] = res1.results[c]["yb"][real]
    yrows = yrows.reshape(N, K, D)

    ln_identity = bool(np.all(ln_w == 1.0) and np.all(ln_b == 0.0))
    nc2 = _build_combine(D, NT, K, ln_identity)
    nc2.finalize()

    in_maps2 = []
    for c in range(N_CORES):
        sl = slice(c * NT, (c + 1) * NT)
        xc = (
            x[sl] + np.einsum("nk,nkd->nd", wts[sl], b2[topk[sl]]).astype(np.float32)
        ).astype(NP_BF16)
        in_maps2.append(
            {"xr": xc, "yg": yrows[sl], "lnw": ln_w, "lnb": ln_b}
        )
    res2 = run_bass_kernel_spmd(nc2, in_maps2, list(range(N_CORES)), trace=trace)

    if trace:
        t1 = res1.exec_time_ns or 0
        t2 = res2.exec_time_ns or 0
        kernel.last_exec_time_ns = t1 + t2

    out = np.empty((N, D), np.float32)
    for c in range(N_CORES):
        out[c * NT : (c + 1) * NT] = res2.results[c]["out"]
    return out


# revision 47
# speedup vs baseline: 1.1759x; 1.1759x over previous
"""MoE block (N=8192, D=1024, H=4096, E=8, top_k=2) on 8 Trainium2 NeuronCores.

Strategy
--------
Expert-parallel FFN + data-parallel combine, two SPMD launches.

Launch 1 (FFN): the 16384 (token, expert) pairs are packed into a uniform
per-core grid of 6 single-expert chunks (5x384 + 1x256 slots = 2176/core).
Chunk capacities are identical on every core (SPMD), and each chunk's
expert weights are shipped per-core in chunk-slot order, so no core ever
re-streams a weight matrix and every matmul2 token tile is full-width.
Both matmuls run in fp8-e4m3 with the TensorEngine's DoubleRow perf mode;
all quantization scales are powers of two folded host-side into b1, the
combine weights, and one constant. Outputs: w_k-scaled y rows in slot
order (bf16).

Host between launches: pure permutation of y rows into token order.

Launch 2 (combine): data-parallel over tokens; per 128-token tile, load
the K=2 weighted expert rows + residual (with b2 folded in host-side),
two adds on Pool/Vector, then LayerNorm.

Measured end-to-end relative error ~8e-3 (gate: 2e-2).
"""

import os
import sys

import numpy as np

for _p in ("/opt/trn_rl_repo", "/root/.axon_site/_ro/trn_rl_repo"):
    if os.path.isdir(_p) and _p not in sys.path:
        sys.path.append(_p)

import ml_dtypes

import concourse.bass as bass
import concourse.mybir as mybir
import concourse.tile as tile
from concourse import bacc
from concourse.bass_utils import run_bass_kernel_spmd

FP8 = mybir.dt.float8e4
BF16 = mybir.dt.bfloat16
F32 = mybir.dt.float32
NP_FP8 = ml_dtypes.float8_e4m3
NP_BF16 = ml_dtypes.bfloat16
DOUBLE_ROW = mybir.MatmulPerfMode.DoubleRow

P = 128          # SBUF partitions
MMAX = 512       # max moving free dim / fp32 PSUM bank
BIG, SMALL = 384, 256   # uniform chunk grid: 5 BIG + 1 SMALL per core
LN_EPS = 1e-5
N_CORES = 8

# fp8 quantization scales (powers of two; e4m3 max finite = 240)
SX = 16.0   # x ~ N(0,1), max |x| ~ 4.5 -> 72
SW1 = 64.0  # w1 ~ U(-1/32, 1/32) -> 2
SH = 16.0   # h = relu(...) max ~ 4 -> 64
SW2 = 64.0  # w2 ~ U(-1/64, 1/64) -> 1
MM1_EVAC_SCALE = SH / (SX * SW1)


# ---------------------------------------------------------------- host routing

def _softmax(z, axis=-1):
    z = z - z.max(axis=axis, keepdims=True)
    ez = np.exp(z)
    return ez / ez.sum(axis=axis, keepdims=True)


def _route(x, gate_w, gate_b, top_k):
    """fp64 gating. Returns topk idx [N,K] and renormalized weights [N,K] f32."""
    logits = x.astype(np.float64) @ gate_w.astype(np.float64).T + gate_b.astype(
        np.float64
    )
    p = _softmax(logits)
    # stable argsort of -p == jax.lax.top_k tie-breaking (lower index first)
    topk = np.argsort(-p, axis=-1, kind="stable")[:, :top_k]
    ps = np.take_along_axis(p, topk, axis=1)
    w = _softmax(ps).astype(np.float32)
    return topk, w


def _pack_experts(loads, n_big, n_small):
    """Per-expert (bigs, smalls) chunk counts covering each load, fitting the
    global supply. Brute force over a small option window per expert."""
    import itertools

    opts = []
    for ld in loads:
        fl = int(ld) // BIG
        o = []
        for a in range(max(0, fl - 2), fl + 3):
            need = int(ld) - a * BIG
            b = 0 if need <= 0 else -(-need // SMALL)
            if b <= n_small:
                o.append((a, b))
        opts.append(o)
    best = None
    for combo in itertools.product(*opts):
        sa = sum(a for a, _ in combo)
        sb = sum(b for _, b in combo)
        if sa <= n_big and sb <= n_small:
            key = (sa + sb, sb)
            if best is None or key < best[0]:
                best = (key, combo)
    assert best is not None, "chunk packing infeasible"
    return best[1]


# ------------------------------------------------------------- device programs

def _build_ffn(D, H, NCH, slot_sizes, offs, CT):
    """Launch 1: uniform expert-chunk FFN. NCH chunks per core."""
    nc = bacc.Bacc()

    nDP = D // (2 * P)    # DoubleRow contraction pairs for matmul1 (4)
    nH = H // P           # h blocks (32)
    nHP = nH // 2         # h pairs for matmul2 contraction (16)
    nDC = (D + MMAX - 1) // MMAX

    xg_d = nc.dram_tensor("xg", [nDP, P, 2, CT], FP8, kind="ExternalInput")
    w1c_d = nc.dram_tensor("w1c", [NCH, nDP, P, 2, H], FP8, kind="ExternalInput")
    w2c_d = nc.dram_tensor("w2c", [NCH, nHP, P, 2, D], FP8, kind="ExternalInput")
    # pre-transposed on host: b1c[p, ci*nH+hb] = b1[expert(ci), hb*P+p]
    b1c_d = nc.dram_tensor("b1c", [P, NCH * nH], F32, kind="ExternalInput")
    wexp_d = nc.dram_tensor("wexp", [CT + P], F32, kind="ExternalInput")
    yb_d = nc.dram_tensor("yb", [CT, D], BF16, kind="ExternalOutput")

    with tile.TileContext(nc) as tc:
        with (
            tc.tile_pool(name="consts", bufs=1) as consts,
            tc.tile_pool(name="w1p", bufs=8) as w1p,
            tc.tile_pool(name="w2p", bufs=28) as w2p,
            tc.tile_pool(name="xgp", bufs=2 * nDP) as xgp,
            tc.tile_pool(name="htp", bufs=6) as htp,
            tc.tile_pool(name="yp", bufs=4) as yp,
            tc.tile_pool(name="sp", bufs=4) as sp,
            tc.tile_pool(name="php", bufs=2, space="PSUM") as php,
            tc.tile_pool(name="pyp", bufs=6, space="PSUM") as pyp,
        ):
            # b1 for all chunks, h-on-partition layout (host pre-transposed:
            # a plain 2D load, not a 26k-descriptor element-wise transpose)
            b1a_t = consts.tile([P, NCH * nH], F32)
            nc.sync.dma_start(out=b1a_t, in_=b1c_d[:, :])

            def load_xg_wx(ci, engs=None):
                off, C = offs[ci], slot_sizes[ci]
                ntt = (C + P - 1) // P
                xg_t = []
                for dp in range(nDP):
                    t = xgp.tile([P, 2, C], FP8, tag="xg", name=f"xg_{ci}_{dp}")
                    # steady state issues from GpSimd: the Sync sequencer is
                    # otherwise the DMA-issue bottleneck (~98% busy)
                    eng = engs[dp % len(engs)] if engs else nc.gpsimd
                    eng.dma_start(out=t, in_=xg_d[dp, :, :, off : off + C])
                    xg_t.append(t)
                wx_t = sp.tile([P, ntt], F32, tag="wx")
                _l = wexp_d[off : off + C]
                nc.gpsimd.dma_start(
                    out=wx_t[:, :],
                    in_=bass.AP(
                        tensor=_l.tensor, offset=_l.offset, ap=[[1, P], [P, ntt]]
                    ),
                )
                return xg_t, wx_t

            def alloc_w1(ci):
                return [
                    w1p.tile([P, 2, H], FP8, tag="w1", name=f"w1_{ci}_{dp}")
                    for dp in range(nDP)
                ]

            def load_w1_part(ci, w1_t, part, eng=None):
                # q-major: all four dp slices of quarter 0 land first, so the
                # first h-blocks' matmuls can start after ~1MB, not ~3.4MB
                q, dp = divmod(part, nDP)
                hq = H // nDP
                eng = eng or (nc.sync if part % 2 else nc.gpsimd)
                eng.dma_start(
                    out=w1_t[dp][:, :, q * hq : (q + 1) * hq],
                    in_=w1c_d[ci, dp, :, :, q * hq : (q + 1) * hq],
                )

            # chunk-0 startup: spread DMA issue over three engine sequencers
            # so descriptor generation isn't serialized on Sync
            startup_engs = [nc.sync, nc.scalar, nc.gpsimd]
            pend_xg = load_xg_wx(0, engs=startup_engs)
            pend_w1 = alloc_w1(0)
            for part in range(nHP):
                load_w1_part(0, pend_w1, part, eng=startup_engs[part % 3])

            for ci in range(NCH):
                off, C = offs[ci], slot_sizes[ci]
                ntt = (C + P - 1) // P
                xg_t, wx_t = pend_xg
                w1_t = pend_w1
                nxt = ci + 1 if ci + 1 < NCH else None
                if nxt is not None:
                    pend_w1 = alloc_w1(nxt)

                w2_t = []
                for hp in range(nHP):
                    t = w2p.tile([P, 2, D], FP8, tag="w2")
                    nc.gpsimd.dma_start(out=t, in_=w2c_d[ci, hp])
                    w2_t.append(t)
                    if nxt is not None:
                        if hp == 0:
                            pend_xg = load_xg_wx(nxt)
                        load_w1_part(nxt, pend_w1, hp)

                py = {}
                for tt in range(ntt):
                    for ch in range(nDC):
                        pt = pyp.tile([P, MMAX], F32, tag="py")
                        py[(tt, ch)] = pt

                ht_t = {}

                def mm2_step(hp, py=py, ht_t=ht_t, w2_t=w2_t, C=C, ntt=ntt):
                    for tt in range(ntt):
                        s = tt * P
                        mw = min(P, C - s)
                        for ch in range(nDC):
                            d0 = ch * MMAX
                            dw = min(MMAX, D - d0)
                            nc.tensor.matmul(
                                py[(tt, ch)][:mw, :dw],
                                ht_t[hp][:, :, s : s + mw],
                                w2_t[hp][:, :, d0 : d0 + dw],
                                start=(hp == 0),
                                stop=(hp == nHP - 1),
                                perf_mode=DOUBLE_ROW,
                            )

                for hb in range(nH):
                    hp, j = hb // 2, hb % 2
                    if j == 0:
                        ht = htp.tile([P, 2, C], FP8, tag="ht")
                        ht_t[hp] = ht
                    ph = php.tile([P, MMAX], F32, tag="ph")
                    for dp in range(nDP):
                        nc.tensor.matmul(
                            ph[:, :C],
                            w1_t[dp][:, :, hb * P : (hb + 1) * P],
                            xg_t[dp][:, :, :],
                            start=(dp == 0),
                            stop=(dp == nDP - 1),
                            perf_mode=DOUBLE_ROW,
                        )
                    nc.scalar.activation(
                        out=ht_t[hp][:, j, :],
                        in_=ph[:, :C],
                        func=mybir.ActivationFunctionType.Relu,
                        bias=b1a_t[:, ci * nH + hb : ci * nH + hb + 1],
                        scale=MM1_EVAC_SCALE,
                    )
                    if j == 1 and hp >= 3:
                        mm2_step(hp - 3)
                mm2_step(nHP - 3)
                mm2_step(nHP - 2)
                mm2_step(nHP - 1)

                # evacuate y on the Vector engine, scaled by the combine
                # weight, straight to slot order (host permutes afterwards)
                for tt in range(ntt):
                    s = tt * P
                    mw = min(P, C - s)
                    yt = yp.tile([P, D], BF16, tag="y")
                    for ch in range(nDC):
                        d0 = ch * MMAX
                        dw = min(MMAX, D - d0)
                        nc.vector.tensor_scalar_mul(
                            yt[:mw, d0 : d0 + dw],
                            py[(tt, ch)][:mw, :dw],
                            wx_t[:mw, tt : tt + 1],
                        )
                    nc.sync.dma_start(
                        out=yb_d[off + s : off + s + mw, :], in_=yt[:mw]
                    )

    return nc


def _build_combine(D, NT, K, ln_identity):
    """Launch 2: residual + top-k combine + LayerNorm, data-parallel."""
    nc = bacc.Bacc()

    xr_d = nc.dram_tensor("xr", [NT, D], BF16, kind="ExternalInput")
    yg_d = nc.dram_tensor("yg", [NT, K, D], BF16, kind="ExternalInput")
    lnw_d = nc.dram_tensor("lnw", [D], F32, kind="ExternalInput")
    lnb_d = nc.dram_tensor("lnb", [D], F32, kind="ExternalInput")
    out_d = nc.dram_tensor("out", [NT, D], F32, kind="ExternalOutput")

    with tile.TileContext(nc) as tc:
        with (
            tc.tile_pool(name="consts", bufs=1) as consts,
            tc.tile_pool(name="cp", bufs=8) as cp,
            tc.tile_pool(name="ygp", bufs=8) as ygp,
            tc.tile_pool(name="sp", bufs=8) as sp,
        ):
            eps_t = consts.tile([P, 1], F32)
            nc.vector.memset(eps_t, LN_EPS)
            if not ln_identity:
                lnw_t = consts.tile([P, D], F32)
                _l = lnw_d[:]
                nc.sync.dma_start(
                    out=lnw_t,
                    in_=bass.AP(
                        tensor=_l.tensor, offset=_l.offset, ap=[[0, P], [1, D]]
                    ),
                )
                lnb_t = consts.tile([P, D], F32)
                _l = lnb_d[:]
                nc.sync.dma_start(
                    out=lnb_t,
                    in_=bass.AP(
                        tensor=_l.tensor, offset=_l.offset, ap=[[0, P], [1, D]]
                    ),
                )

            for t in range(NT // P):
                r0 = t * P
                xr_t = cp.tile([P, D], BF16, tag="xr")
                nc.scalar.dma_start(out=xr_t, in_=xr_d[r0 : r0 + P, :])
                yg = ygp.tile([P, K, D], BF16, tag="yg")
                nc.gpsimd.dma_start(out=yg, in_=yg_d[r0 : r0 + P])
                # adds split by half-columns and pipelined across Pool/DVE,
                # with each half's LN stats emitted as soon as it's ready
                ys = cp.tile([P, D], F32, tag="ys")
                acc = cp.tile([P, D], F32, tag="acc")
                hD = D // 2
                nsub = (D + 511) // 512
                assert nsub == 2 and hD == 512
                st = sp.tile([P, nsub, 6], F32, tag="st")
                for s, h0 in enumerate((0, hD)):
                    nc.gpsimd.tensor_add(
                        ys[:, h0 : h0 + hD],
                        yg[:, 0, h0 : h0 + hD],
                        yg[:, 1, h0 : h0 + hD],
                    )
                    nc.vector.tensor_add(
                        acc[:, h0 : h0 + hD],
                        xr_t[:, h0 : h0 + hD],
                        ys[:, h0 : h0 + hD],
                    )
                    nc.vector.bn_stats(
                        out=st[:, s, :], in_=acc[:, h0 : h0 + hD]
                    )
                mv = sp.tile([P, 2], F32, tag="mv")
                nc.vector.bn_aggr(out=mv, in_=st)
                nc.scalar.activation(
                    out=mv[:, 1:2],
                    in_=mv[:, 1:2],
                    func=mybir.ActivationFunctionType.Sqrt,
                    bias=eps_t[:, 0:1],
                )
                nc.vector.reciprocal(out=mv[:, 1:2], in_=mv[:, 1:2])
                nb = sp.tile([P, 1], F32, tag="nb")
                nc.vector.tensor_scalar(
                    out=nb,
                    in0=mv[:, 0:1],
                    scalar1=mv[:, 1:2],
                    scalar2=-1.0,
                    op0=mybir.AluOpType.mult,
                    op1=mybir.AluOpType.mult,
                )
                # final scale+shift on DVE (one fewer cross-engine hop than
                # the ACT Identity path)
                nc.vector.tensor_scalar(
                    out=acc,
                    in0=acc,
                    scalar1=mv[:, 1:2],
                    scalar2=nb[:, 0:1],
                    op0=mybir.AluOpType.mult,
                    op1=mybir.AluOpType.add,
                )
                if not ln_identity:
                    nc.vector.tensor_mul(acc, acc, lnw_t)
                    nc.vector.tensor_add(acc, acc, lnb_t)
                nc.sync.dma_start(out=out_d[r0 : r0 + P, :], in_=acc)

    return nc


# ----------------------------------------------------------------- entrypoint

def kernel(x, gate_w, gate_b, w1, b1, w2, b2, ln_w, ln_b, top_k):
    x = np.asarray(x, np.float32)
    w1 = np.asarray(w1, np.float32)
    b1 = np.asarray(b1, np.float32)
    w2 = np.asarray(w2, np.float32)
    b2 = np.asarray(b2, np.float32)
    ln_w = np.asarray(ln_w, np.float32)
    ln_b = np.asarray(ln_b, np.float32)
    K = int(top_k)

    N, D = x.shape
    E, H, _ = w1.shape
    NT = N // N_CORES
    assert N % (N_CORES * P) == 0 and D % (2 * P) == 0 and H % (2 * P) == 0

    topk, wts = _route(x, np.asarray(gate_w, np.float32), np.asarray(gate_b, np.float32), K)

    # ---- pack (token, expert) pairs into the uniform chunk grid
    loads = np.bincount(topk.ravel(), minlength=E)
    n_big, n_small = 5 * N_CORES, N_CORES
    ab = _pack_experts(loads, n_big, n_small)

    slot_sizes = [BIG] * 5 + [SMALL]
    offs = np.concatenate([[0], np.cumsum(slot_sizes)]).astype(np.int64)
    CT = int(offs[-1])
    NCH = len(slot_sizes)

    # global chunk list: (expert, pair_lo, pair_hi) in expert-major order;
    # -1 expert = dead filler chunk
    big_list, small_list = [], []
    pair_tok = [np.where((topk == e).any(axis=1))[0] for e in range(E)]
    pair_kk = [np.argmax(topk[pt] == e, axis=1) for e, pt in enumerate(pair_tok)]
    for e in range(E):
        a_e, b_e = ab[e]
        pos = 0
        for _ in range(a_e):
            big_list.append((e, pos, min(pos + BIG, int(loads[e]))))
            pos = min(pos + BIG, int(loads[e]))
        for _ in range(b_e):
            small_list.append((e, pos, min(pos + SMALL, int(loads[e]))))
            pos = min(pos + SMALL, int(loads[e]))
        assert pos >= loads[e]
    big_list += [(-1, 0, 0)] * (n_big - len(big_list))
    small_list += [(-1, 0, 0)] * (n_small - len(small_list))

    nDP, nHP = D // (2 * P), H // (2 * P)
    w1t = np.ascontiguousarray(
        (w1.transpose(0, 2, 1) * SW1)
        .astype(NP_FP8)
        .reshape(E, nDP, 2, P, H)
        .transpose(0, 1, 3, 2, 4)
    )
    w2t = np.ascontiguousarray(
        (w2.transpose(0, 2, 1) * SW2)
        .astype(NP_FP8)
        .reshape(E, nHP, 2, P, D)
        .transpose(0, 1, 3, 2, 4)
    )
    b1s = (b1 * SH).astype(np.float32)
    xq_all = (x * SX).astype(NP_FP8)  # [N, D]

    in_maps1 = []
    # permutation: for each core/slot -> destination row (token*K + k)
    dest_rows = []
    for c in range(N_CORES):
        core_chunks = big_list[5 * c : 5 * c + 5] + [small_list[c]]
        xgbuf = np.zeros((nDP, P, 2, CT), NP_FP8)
        w1c = np.zeros((NCH, nDP, P, 2, H), NP_FP8)
        w2c = np.zeros((NCH, nHP, P, 2, D), NP_FP8)
        b1c = np.zeros((NCH, H), np.float32)  # transposed to [P, NCH*nH] below
        wexp = np.zeros(CT + P, np.float32)
        drow = np.full(CT, -1, np.int64)
        for ci, (e, lo, hi) in enumerate(core_chunks):
            if e < 0 or hi <= lo:
                continue
            toks = pair_tok[e][lo:hi]
            kks = pair_kk[e][lo:hi]
            off = int(offs[ci])
            n_p = hi - lo
            xgbuf[:, :, :, off : off + n_p] = (
                xq_all[toks].T.reshape(nDP, 2, P, n_p).transpose(0, 2, 1, 3)
            )
            w1c[ci] = w1t[e]
            w2c[ci] = w2t[e]
            b1c[ci] = b1s[e]
            wexp[off : off + n_p] = wts[toks, kks] / (SH * SW2)
            drow[off : off + n_p] = toks * K + kks
        dest_rows.append(drow)
        nH = H // P
        b1ct = np.ascontiguousarray(
            b1c.reshape(NCH, nH, P).transpose(2, 0, 1).reshape(P, NCH * nH)
        )
        in_maps1.append(
            {"xg": xgbuf, "w1c": w1c, "w2c": w2c, "b1c": b1ct, "wexp": wexp}
        )

    nc1 = _build_ffn(D, H, NCH, slot_sizes, offs, CT)
    nc1.finalize()

    trace = os.environ.get("MOE_KERNEL_TRACE", "0") == "1"
    res1 = run_bass_kernel_spmd(nc1, in_maps1, list(range(N_CORES)), trace=trace)

    # ---- host: permute y rows into token order (pure data movement)
    yrows = np.zeros((N * K, D), NP_BF16)
    for c in range(N_CORES):
        dr = dest_rows[c]
        real = dr >= 0
        yrows[dr[real]] = res1.results[c]["yb"][real]
    yrows = yrows.reshape(N, K, D)

    ln_identity = bool(np.all(ln_w == 1.0) and np.all(ln_b == 0.0))
    nc2 = _build_combine(D, NT, K, ln_identity)
    nc2.finalize()

    in_maps2 = []
    for c in range(N_CORES):
        sl = slice(c * NT, (c + 1) * NT)
        xc = (
            x[sl] + np.einsum("nk,nkd->nd", wts[sl], b2[topk[sl]]).astype(np.float32)
        ).astype(NP_BF16)
        in_maps2.append(
            {"xr": xc, "yg": yrows[sl], "lnw": ln_w, "lnb": ln_b}
        )
    res2 = run_bass_kernel_spmd(nc2, in_maps2, list(range(N_CORES)), trace=trace)

    if trace:
        t1 = res1.exec_time_ns or 0
        t2 = res2.exec_time_ns or 0
        kernel.last_exec_time_ns = t1 + t2

    out = np.empty((N, D), np.float32)
    for c in range(N_CORES):
        out[c * NT : (c + 1) * NT] = res2.results[c]["out"]
    return out
